# revision 49
# baseline (speedup 1.0000x reference)
"""Trainium2 Bass kernel: AnalyticHashDepthwiseConv2d.

Depthwise 7x7 conv (stride 1, SAME padding) on x[16, 512, 96, 96] f32.
Weights come from a hashed codebook gather: weight[c,kh,kw] =
codebook[idx(c,kh,kw)] * sign(c,kh,kw); bias added per channel.

Strategy
--------
Channel-shard across 8 NeuronCores (64 channels each; depthwise conv has
no cross-channel mixing).  On each core the conv is mapped to the
TensorEngine as banded-Toeplitz matmuls over H:

  out[h_out, (n,w)] = sum_kw sum_h_in T_{c,kw}[h_in, h_out] * x[h_in, (n, w+kw-3)]

with T_{c,kw}[h_in, h_out] = weight[c, h_in-h_out+3, kw] on a 7-diagonal
band (edge truncation of the band implements zero padding in H for free).
The kw shifts are free-dim offsets into a zero-padded SBUF x tile; the 7
kw terms accumulate in PSUM.  Per (channel, n-chunk): 7 matmuls
[K=96] x [N=384].  Compute in bf16 (PSUM accumulates f32), bias fused
into the PSUM->SBUF copy on the ScalarEngine.
"""

import sys

sys.path.insert(0, "/opt/trn_rl_repo")

from contextlib import ExitStack

import numpy as np
import ml_dtypes

import concourse.bass as bass
import concourse.mybir as mybir
from concourse.bass_utils import run_bass_kernel_spmd

# ---- problem constants (hardcoded; must match the reference) ----
DW_HASH_CH = 1337
DW_HASH_KH = 7919
DW_HASH_KW = 2971
DW_HASH_LAYER = 104729
SIGN_HASH_A = 4099
SIGN_HASH_B = 6151
SIGN_HASH_C = 14887

N, C, H, W = 16, 512, 96, 96
KH, KW = 7, 7
PAD = 3
CODEBOOK_SIZE = 4096
LAYER_ID = 3

NCORES = 8
CPC = C // NCORES          # channels per core
WPAD = W + 2 * PAD         # zero-padded w extent in SBUF
NCHUNK = 4                 # images per matmul chunk
NB = N // NCHUNK           # chunks per channel
FREE = NCHUNK * W          # matmul free size (384 <= 512 psum bank)

BF16 = mybir.dt.bfloat16
F32 = mybir.dt.float32
NP_BF16 = ml_dtypes.bfloat16


def _hash_weight(codebook: np.ndarray) -> np.ndarray:
    """weight[c, kh, kw] from the hashed codebook gather + sign flip."""
    ch = np.arange(C).reshape(C, 1, 1)
    kh = np.arange(KH).reshape(1, KH, 1)
    kw = np.arange(KW).reshape(1, 1, KW)
    idx = (ch * DW_HASH_CH + kh * DW_HASH_KH + kw * DW_HASH_KW
           + LAYER_ID * DW_HASH_LAYER) % CODEBOOK_SIZE
    bits = (ch * SIGN_HASH_A + kh * SIGN_HASH_B + kw * SIGN_HASH_C
            + LAYER_ID * (SIGN_HASH_A + 29)) % 2
    signs = bits.astype(np.float32) * 2.0 - 1.0
    return codebook.astype(np.float32)[idx] * signs  # [C, KH, KW]


def _toeplitz_simple(weight: np.ndarray) -> np.ndarray:
    """Banded stationaries T[c, h_in, kw, h_out] = weight[c, h_in-h_out+3, kw]."""
    T = np.zeros((C, H, KW, H), dtype=np.float32)
    for kh in range(KH):
        for hi in range(H):
            ho = hi - (kh - PAD)
            if 0 <= ho < H:
                T[:, hi, :, ho] = weight[:, kh, :]
    return T


def _build_nc() -> bass.Bass:
    """Raw-bass builder: explicit per-engine streams with standalone
    wait_ge instructions (walrus only fits 2 sync commands — waits plus
    updates — per TPB instruction, so waits must be their own queue
    entries; every DMA/compute instruction carries at most one then_inc).

    DMA completions on a queue are NOT ordered across in-flight DMAs, so
    each ring slot / wall chunk gets its OWN semaphore: a wait then names
    exactly the transfer it needs.  Engine-sem increments (s_pe, s_cp) are
    engine-FIFO and stay single counters.

    Protocol (per channel c):
      SYNC:   [c>=XD] wait s_pe>=c-XD+1 ; dma x(c)->xring[c%XD]
              (+16 s_xs[c%XD]); wall preloaded in 8 chunks (+16 s_ws[wc])
      SCALAR: wait s_cp>=c+1 ; dma oring[c%OD]->out(c) (+16 s_outs[c%OD])
      PE:     [c%8==0] wait s_ws[c//8]>=16; wait s_xs[c%XD]>=16(c//XD+1)
              [c>=2] wait s_cp>=c-1 ; 28 matmuls into ps[c%2]; last +1 s_pe
      DVE:    wait s_pe>=c+1; [c>=OD] wait s_outs[c%OD]>=16(c//OD)
              copy ps[c%2] -> oring[c%OD] (f32->bf16), +1 s_cp
    """
    nc = bass.Bass()
    # host packs x as [c, h, n, w_pad] and wt as [h, c, kw, m]; out comes
    # back as [c, h, n, w] — all DMAs fully contiguous per partition row.
    xs = nc.declare_dram_parameter("xs", [CPC, H, N, WPAD], BF16, isOutput=False)
    wt = nc.declare_dram_parameter("wt", [H, CPC, KW, H], BF16, isOutput=False)
    out = nc.declare_dram_parameter("out", [CPC, H, N, W], BF16, isOutput=True)

    BANK = 512   # f32 elems per PSUM bank
    XD = 8       # x-ring depth
    OD = 8       # out-ring depth
    WCHUNK = 8   # channels per wall-preload chunk
    NWC = CPC // WCHUNK
    WCOLS = KW * H

    with ExitStack() as ctx:
        wall = ctx.enter_context(nc.sbuf_tensor("wall", [H, CPC * KW * H], BF16))
        xring = [
            ctx.enter_context(nc.sbuf_tensor(f"xb{i}", [H, N * WPAD], BF16))
            for i in range(XD)
        ]
        oring = [
            ctx.enter_context(nc.sbuf_tensor(f"ob{i}", [H, N * W], BF16))
            for i in range(OD)
        ]
        psums = [
            ctx.enter_context(nc.psum_tensor(f"ps{i}", [H, NB * BANK], F32))
            for i in range(2)
        ]
        s_xs = [ctx.enter_context(nc.semaphore(f"s_x{i}")) for i in range(XD)]
        s_ws = [ctx.enter_context(nc.semaphore(f"s_w{i}")) for i in range(NWC)]
        s_outs = [ctx.enter_context(nc.semaphore(f"s_o{i}")) for i in range(OD)]
        s_pe = ctx.enter_context(nc.semaphore("s_pe"))
        s_cp = ctx.enter_context(nc.semaphore("s_cp"))
        block = ctx.enter_context(nc.Block())

        @block.sync
        def _(sync):
            # interleave early x tiles with the wall chunks so PE has a
            # steady diet from channel 0 onward
            for cl in range(CPC):
                if cl >= XD:
                    sync.wait_ge(s_pe, NB * (cl - XD + 1))
                sync.dma_start(
                    xring[cl % XD][:], xs[cl].rearrange("p n w -> p (n w)")
                ).then_inc(s_xs[cl % XD], 16)
                if cl < NWC:
                    wc = cl
                    sync.dma_start(
                        wall[:, wc * WCHUNK * WCOLS:(wc + 1) * WCHUNK * WCOLS],
                        wt[:, wc * WCHUNK:(wc + 1) * WCHUNK].rearrange(
                            "h c k m -> h (c k m)"
                        ),
                    ).then_inc(s_ws[wc], 16)
            for i in range(OD):
                sync.wait_ge(s_outs[i], 16 * (CPC // OD))

        @block.scalar
        def _(scalar):
            for cl in range(CPC):
                scalar.wait_ge(s_cp, NB * (cl + 1))
                scalar.dma_start(
                    out[cl].rearrange("p n w -> p (n w)"), oring[cl % OD][:]
                ).then_inc(s_outs[cl % OD], 16)

        @block.tensor
        def _(tensor):
            wallv = wall[:].rearrange("p (c k m) -> p c k m", c=CPC, k=KW)
            for cl in range(CPC):
                if cl % WCHUNK == 0:
                    tensor.wait_ge(s_ws[cl // WCHUNK], 16)
                tensor.wait_ge(s_xs[cl % XD], 16 * (cl // XD + 1))
                xv = xring[cl % XD][:].rearrange("p (n w) -> p n w", w=WPAD)
                ps = psums[cl % 2]
                for nb in range(NB):
                    if cl >= 2:
                        # psum chunk reused from (cl-2, nb): its copy done?
                        tensor.wait_ge(s_cp, NB * (cl - 2) + nb + 1)
                    for kw in range(KW):
                        rhs = xv[:, nb * NCHUNK:(nb + 1) * NCHUNK, kw:kw + W]
                        mm = nc.tensor.matmul(
                            ps[:, nb * BANK:nb * BANK + FREE],
                            wallv[:, cl, kw, :],
                            rhs,
                            start=(kw == 0),
                            stop=(kw == KW - 1),
                        )
                    mm.then_inc(s_pe, 1)

        @block.vector
        def _(vector):
            for cl in range(CPC):
                if cl >= OD:
                    vector.wait_ge(s_outs[cl % OD], 16 * (cl // OD))
                for nb in range(NB):
                    vector.wait_ge(s_pe, NB * cl + nb + 1)
                    src = psums[cl % 2][
                        :, nb * BANK:nb * BANK + FREE
                    ].rearrange("p (n w) -> p n w", w=W)
                    dst = oring[cl % OD][
                        :, nb * FREE:(nb + 1) * FREE
                    ].rearrange("p (n w) -> p n w", w=W)
                    nc.vector.tensor_copy(dst, src).then_inc(s_cp, 1)

    return nc


_NC_CACHE = None


def _get_nc() -> bass.Bass:
    global _NC_CACHE
    if _NC_CACHE is None:
        _NC_CACHE = _build_nc()
    return _NC_CACHE


def _run(x, codebook, bias, trace=False, **spmd_kwargs):
    weight = _hash_weight(np.asarray(codebook))
    T = _toeplitz_simple(weight).astype(NP_BF16)  # [C, H, KW, H]
    x = np.asarray(x)
    bias = np.asarray(bias).astype(np.float32)

    # pack x as [C, H, N, WPAD] so device DMA rows are contiguous
    x_bf = np.zeros((C, H, N, WPAD), dtype=NP_BF16)
    x_bf[:, :, :, PAD:PAD + W] = x.astype(NP_BF16).transpose(1, 2, 0, 3)
    in_maps = []
    for i in range(NCORES):
        c0 = i * CPC
        in_maps.append(
            {
                "xs": np.ascontiguousarray(x_bf[c0:c0 + CPC]),
                # wall layout [h, c, kw, m]
                "wt": np.ascontiguousarray(
                    T[c0:c0 + CPC].transpose(1, 0, 2, 3)
                ),
            }
        )

    nc = _get_nc()
    res = run_bass_kernel_spmd(
        nc, in_maps, core_ids=list(range(NCORES)), trace=trace, **spmd_kwargs
    )
    # device out is [CPC, H, N, W] bf16 per core -> [N, C, H, W] f32
    out = np.concatenate(
        [np.asarray(r["out"]) for r in res.results], axis=0
    ).transpose(2, 0, 1, 3).astype(np.float32)
    if np.any(bias):
        out = out + bias[None, :, None, None]
    return out, res


def kernel(x, codebook, bias):
    out, _ = _run(x, codebook, bias, trace=False)
    return out


if __name__ == "__main__":
    # smoke test: tiny numeric check of the toeplitz construction vs direct conv
    rng = np.random.default_rng(0)
    cb = (rng.standard_normal(CODEBOOK_SIZE) * 0.01).astype(np.float32)
    w = _hash_weight(cb)
    T = _toeplitz_simple(w)
    # direct conv of one channel, one image column
    xs = rng.standard_normal((C, H)).astype(np.float32)
    c = 123
    ref = np.zeros(H)
    for ho in range(H):
        for kh in range(KH):
            hi = ho + kh - PAD
            if 0 <= hi < H:
                ref[ho] += w[c, kh, 2] * xs[c, hi]
    got = T[c, :, 2, :].T @ xs[c]
    print("toeplitz check:", np.abs(ref - got).max())


# revision 51
# speedup vs baseline: 1.1725x; 1.1725x over previous
"""Trainium2 Bass kernel: AnalyticHashDepthwiseConv2d.

Depthwise 7x7 conv (stride 1, SAME padding) on x[16, 512, 96, 96] f32.
Weights come from a hashed codebook gather: weight[c,kh,kw] =
codebook[idx(c,kh,kw)] * sign(c,kh,kw); bias added per channel.

Strategy
--------
Channel-shard across 8 NeuronCores (64 channels each; depthwise conv has
no cross-channel mixing).  On each core the conv is mapped to the
TensorEngine as banded-Toeplitz matmuls over H:

  out[h_out, (n,w)] = sum_kw sum_h_in T_{c,kw}[h_in, h_out] * x[h_in, (n, w+kw-3)]

with T_{c,kw}[h_in, h_out] = weight[c, h_in-h_out+3, kw] on a 7-diagonal
band (edge truncation of the band implements zero padding in H for free).
The kw shifts are free-dim offsets into a zero-padded SBUF x tile; the 7
kw terms accumulate in PSUM.  Per (channel, n-chunk): 7 matmuls
[K=96] x [N=384].  Compute in bf16 (PSUM accumulates f32), bias fused
into the PSUM->SBUF copy on the ScalarEngine.
"""

import sys

sys.path.insert(0, "/opt/trn_rl_repo")

from contextlib import ExitStack

import numpy as np
import ml_dtypes

import concourse.bass as bass
import concourse.mybir as mybir
from concourse.bass_utils import run_bass_kernel_spmd

# ---- problem constants (hardcoded; must match the reference) ----
DW_HASH_CH = 1337
DW_HASH_KH = 7919
DW_HASH_KW = 2971
DW_HASH_LAYER = 104729
SIGN_HASH_A = 4099
SIGN_HASH_B = 6151
SIGN_HASH_C = 14887

N, C, H, W = 16, 512, 96, 96
KH, KW = 7, 7
PAD = 3
CODEBOOK_SIZE = 4096
LAYER_ID = 3

NCORES = 8
CPC = C // NCORES          # channels per core
WPAD = W + 2 * PAD         # zero-padded w extent in SBUF
NCHUNK = 4                 # images per matmul chunk
NB = N // NCHUNK           # chunks per channel
FREE = NCHUNK * W          # matmul free size (384 <= 512 psum bank)

BF16 = mybir.dt.bfloat16
F32 = mybir.dt.float32
NP_BF16 = ml_dtypes.bfloat16


def _hash_weight(codebook: np.ndarray) -> np.ndarray:
    """weight[c, kh, kw] from the hashed codebook gather + sign flip."""
    ch = np.arange(C).reshape(C, 1, 1)
    kh = np.arange(KH).reshape(1, KH, 1)
    kw = np.arange(KW).reshape(1, 1, KW)
    idx = (ch * DW_HASH_CH + kh * DW_HASH_KH + kw * DW_HASH_KW
           + LAYER_ID * DW_HASH_LAYER) % CODEBOOK_SIZE
    bits = (ch * SIGN_HASH_A + kh * SIGN_HASH_B + kw * SIGN_HASH_C
            + LAYER_ID * (SIGN_HASH_A + 29)) % 2
    signs = bits.astype(np.float32) * 2.0 - 1.0
    return codebook.astype(np.float32)[idx] * signs  # [C, KH, KW]


def _toeplitz_simple(weight: np.ndarray) -> np.ndarray:
    """Banded stationaries T[c, h_in, kw, h_out] = weight[c, h_in-h_out+3, kw]."""
    T = np.zeros((C, H, KW, H), dtype=np.float32)
    for kh in range(KH):
        for hi in range(H):
            ho = hi - (kh - PAD)
            if 0 <= ho < H:
                T[:, hi, :, ho] = weight[:, kh, :]
    return T


def _build_nc() -> bass.Bass:
    """Raw-bass builder: explicit per-engine streams with standalone
    wait_ge instructions (walrus only fits 2 sync commands — waits plus
    updates — per TPB instruction, so waits must be their own queue
    entries; every DMA/compute instruction carries at most one then_inc).

    DMA completions on a queue are NOT ordered across in-flight DMAs, so
    each ring slot / wall chunk gets its OWN semaphore: a wait then names
    exactly the transfer it needs.  Engine-sem increments (s_pe, s_cp) are
    engine-FIFO and stay single counters.

    Protocol (per channel c):
      SYNC:   [c>=XD] wait s_pe>=c-XD+1 ; dma x(c)->xring[c%XD]
              (+16 s_xs[c%XD]); wall preloaded in 8 chunks (+16 s_ws[wc])
      SCALAR: wait s_cp>=c+1 ; dma oring[c%OD]->out(c) (+16 s_outs[c%OD])
      PE:     [c%8==0] wait s_ws[c//8]>=16; wait s_xs[c%XD]>=16(c//XD+1)
              [c>=2] wait s_cp>=c-1 ; 28 matmuls into ps[c%2]; last +1 s_pe
      DVE:    wait s_pe>=c+1; [c>=OD] wait s_outs[c%OD]>=16(c//OD)
              copy ps[c%2] -> oring[c%OD] (f32->bf16), +1 s_cp
    """
    nc = bass.Bass()
    # host packs x as [c, h, n, w_pad] and wt as [h, c, kw, m]; out comes
    # back as [c, h, n, w] — all DMAs fully contiguous per partition row.
    xs = nc.declare_dram_parameter("xs", [CPC, H, N, WPAD], BF16, isOutput=False)
    wt = nc.declare_dram_parameter("wt", [H, CPC, KW, H], BF16, isOutput=False)
    out = nc.declare_dram_parameter("out", [CPC, H, N, W], BF16, isOutput=True)

    BANK = 512   # f32 elems per PSUM bank
    XD = 8       # x-ring depth
    OD = 8       # out-ring depth
    WCHUNK = 8   # channels per wall-preload chunk
    NWC = CPC // WCHUNK
    WCOLS = KW * H

    with ExitStack() as ctx:
        wall = ctx.enter_context(nc.sbuf_tensor("wall", [H, CPC * KW * H], BF16))
        xring = [
            ctx.enter_context(nc.sbuf_tensor(f"xb{i}", [H, N * WPAD], BF16))
            for i in range(XD)
        ]
        oring = [
            ctx.enter_context(nc.sbuf_tensor(f"ob{i}", [H, N * W], BF16))
            for i in range(OD)
        ]
        psums = [
            ctx.enter_context(nc.psum_tensor(f"ps{i}", [H, NB * BANK], F32))
            for i in range(2)
        ]
        s_xs = [ctx.enter_context(nc.semaphore(f"s_x{i}")) for i in range(XD)]
        s_ws = [ctx.enter_context(nc.semaphore(f"s_w{i}")) for i in range(NWC)]
        s_outs = [ctx.enter_context(nc.semaphore(f"s_o{i}")) for i in range(OD)]
        s_pe = ctx.enter_context(nc.semaphore("s_pe"))
        s_cp = ctx.enter_context(nc.semaphore("s_cp"))
        block = ctx.enter_context(nc.Block())

        @block.sync
        def _(sync):
            # interleave early x tiles with the wall chunks so PE has a
            # steady diet from channel 0 onward
            for cl in range(CPC):
                if cl >= XD:
                    sync.wait_ge(s_pe, cl - XD + 1)
                sync.dma_start(
                    xring[cl % XD][:], xs[cl].rearrange("p n w -> p (n w)")
                ).then_inc(s_xs[cl % XD], 16)
                if cl < NWC:
                    wc = cl
                    sync.dma_start(
                        wall[:, wc * WCHUNK * WCOLS:(wc + 1) * WCHUNK * WCOLS],
                        wt[:, wc * WCHUNK:(wc + 1) * WCHUNK].rearrange(
                            "h c k m -> h (c k m)"
                        ),
                    ).then_inc(s_ws[wc], 16)
            for i in range(OD):
                sync.wait_ge(s_outs[i], 16 * (CPC // OD))

        @block.scalar
        def _(scalar):
            for cl in range(CPC):
                scalar.wait_ge(s_cp, cl + 1)
                scalar.dma_start(
                    out[cl].rearrange("p n w -> p (n w)"), oring[cl % OD][:]
                ).then_inc(s_outs[cl % OD], 16)

        @block.tensor
        def _(tensor):
            wallv = wall[:].rearrange("p (c k m) -> p c k m", c=CPC, k=KW)
            for cl in range(CPC):
                if cl % WCHUNK == 0:
                    tensor.wait_ge(s_ws[cl // WCHUNK], 16)
                tensor.wait_ge(s_xs[cl % XD], 16 * (cl // XD + 1))
                if cl >= 2:
                    tensor.wait_ge(s_cp, cl - 1)
                xv = xring[cl % XD][:].rearrange("p (n w) -> p n w", w=WPAD)
                ps = psums[cl % 2]
                for nb in range(NB):
                    for kw in range(KW):
                        rhs = xv[:, nb * NCHUNK:(nb + 1) * NCHUNK, kw:kw + W]
                        mm = nc.tensor.matmul(
                            ps[:, nb * BANK:nb * BANK + FREE],
                            wallv[:, cl, kw, :],
                            rhs,
                            start=(kw == 0),
                            stop=(kw == KW - 1),
                        )
                mm.then_inc(s_pe, 1)

        @block.vector
        def _(vector):
            for cl in range(CPC):
                vector.wait_ge(s_pe, cl + 1)
                if cl >= OD:
                    vector.wait_ge(s_outs[cl % OD], 16 * (cl // OD))
                src = psums[cl % 2][:].rearrange("p (n b) -> p n b", b=BANK)[
                    :, :, 0:FREE
                ]
                dst = oring[cl % OD][:].rearrange("p (n w) -> p n w", w=FREE)
                nc.vector.tensor_copy(dst, src).then_inc(s_cp, 1)

    return nc


_NC_CACHE = None


def _get_nc() -> bass.Bass:
    global _NC_CACHE
    if _NC_CACHE is None:
        _NC_CACHE = _build_nc()
    return _NC_CACHE


def _run(x, codebook, bias, trace=False, **spmd_kwargs):
    weight = _hash_weight(np.asarray(codebook))
    T = _toeplitz_simple(weight).astype(NP_BF16)  # [C, H, KW, H]
    x = np.asarray(x)
    bias = np.asarray(bias).astype(np.float32)

    # pack x as [C, H, N, WPAD] so device DMA rows are contiguous
    x_bf = np.zeros((C, H, N, WPAD), dtype=NP_BF16)
    x_bf[:, :, :, PAD:PAD + W] = x.astype(NP_BF16).transpose(1, 2, 0, 3)
    in_maps = []
    for i in range(NCORES):
        c0 = i * CPC
        in_maps.append(
            {
                "xs": np.ascontiguousarray(x_bf[c0:c0 + CPC]),
                # wall layout [h, c, kw, m]
                "wt": np.ascontiguousarray(
                    T[c0:c0 + CPC].transpose(1, 0, 2, 3)
                ),
            }
        )

    nc = _get_nc()
    res = run_bass_kernel_spmd(
        nc, in_maps, core_ids=list(range(NCORES)), trace=trace, **spmd_kwargs
    )
    # device out is [CPC, H, N, W] bf16 per core -> [N, C, H, W] f32
    out = np.concatenate(
        [np.asarray(r["out"]) for r in res.results], axis=0
    ).transpose(2, 0, 1, 3).astype(np.float32)
    if np.any(bias):
        out = out + bias[None, :, None, None]
    return out, res


def kernel(x, codebook, bias):
    out, _ = _run(x, codebook, bias, trace=False)
    return out


if __name__ == "__main__":
    # smoke test: tiny numeric check of the toeplitz construction vs direct conv
    rng = np.random.default_rng(0)
    cb = (rng.standard_normal(CODEBOOK_SIZE) * 0.01).astype(np.float32)
    w = _hash_weight(cb)
    T = _toeplitz_simple(w)
    # direct conv of one channel, one image column
    xs = rng.standard_normal((C, H)).astype(np.float32)
    c = 123
    ref = np.zeros(H)
    for ho in range(H):
        for kh in range(KH):
            hi = ho + kh - PAD
            if 0 <= hi < H:
                ref[ho] += w[c, kh, 2] * xs[c, hi]
    got = T[c, :, 2, :].T @ xs[c]
    print("toeplitz check:", np.abs(ref - got).max())


# revision 52
# speedup vs baseline: 1.1805x; 1.0068x over previous
"""Trainium2 Bass kernel: AnalyticHashDepthwiseConv2d.

Depthwise 7x7 conv (stride 1, SAME padding) on x[16, 512, 96, 96] f32.
Weights come from a hashed codebook gather: weight[c,kh,kw] =
codebook[idx(c,kh,kw)] * sign(c,kh,kw); bias added per channel.

Strategy
--------
Channel-shard across 8 NeuronCores (64 channels each; depthwise conv has
no cross-channel mixing).  On each core the conv is mapped to the
TensorEngine as banded-Toeplitz matmuls over H:

  out[h_out, (n,w)] = sum_kw sum_h_in T_{c,kw}[h_in, h_out] * x[h_in, (n, w+kw-3)]

with T_{c,kw}[h_in, h_out] = weight[c, h_in-h_out+3, kw] on a 7-diagonal
band (edge truncation of the band implements zero padding in H for free).
The kw shifts are free-dim offsets into a zero-padded SBUF x tile; the 7
kw terms accumulate in PSUM.  Per (channel, n-chunk): 7 matmuls
[K=96] x [N=384].  Compute in bf16 (PSUM accumulates f32); the tiny
codebook gather, Toeplitz construction, layout packing, and (zero) bias
add run on the host.
"""

import sys

sys.path.insert(0, "/opt/trn_rl_repo")

from contextlib import ExitStack

import numpy as np
import ml_dtypes

import concourse.bass as bass
import concourse.mybir as mybir
from concourse.bass_utils import run_bass_kernel_spmd

# ---- problem constants (hardcoded; must match the reference) ----
DW_HASH_CH = 1337
DW_HASH_KH = 7919
DW_HASH_KW = 2971
DW_HASH_LAYER = 104729
SIGN_HASH_A = 4099
SIGN_HASH_B = 6151
SIGN_HASH_C = 14887

N, C, H, W = 16, 512, 96, 96
KH, KW = 7, 7
PAD = 3
CODEBOOK_SIZE = 4096
LAYER_ID = 3

NCORES = 8
CPC = C // NCORES          # channels per core
WPAD = W + 2 * PAD         # zero-padded w extent in SBUF
NCHUNK = 4                 # images per matmul chunk
NB = N // NCHUNK           # chunks per channel
FREE = NCHUNK * W          # matmul free size (384 <= 512 psum bank)

BF16 = mybir.dt.bfloat16
F32 = mybir.dt.float32
NP_BF16 = ml_dtypes.bfloat16


def _hash_weight(codebook: np.ndarray) -> np.ndarray:
    """weight[c, kh, kw] from the hashed codebook gather + sign flip."""
    ch = np.arange(C).reshape(C, 1, 1)
    kh = np.arange(KH).reshape(1, KH, 1)
    kw = np.arange(KW).reshape(1, 1, KW)
    idx = (ch * DW_HASH_CH + kh * DW_HASH_KH + kw * DW_HASH_KW
           + LAYER_ID * DW_HASH_LAYER) % CODEBOOK_SIZE
    bits = (ch * SIGN_HASH_A + kh * SIGN_HASH_B + kw * SIGN_HASH_C
            + LAYER_ID * (SIGN_HASH_A + 29)) % 2
    signs = bits.astype(np.float32) * 2.0 - 1.0
    return codebook.astype(np.float32)[idx] * signs  # [C, KH, KW]


def _toeplitz_simple(weight: np.ndarray) -> np.ndarray:
    """Banded stationaries T[c, h_in, kw, h_out] = weight[c, h_in-h_out+3, kw]."""
    T = np.zeros((C, H, KW, H), dtype=np.float32)
    for kh in range(KH):
        for hi in range(H):
            ho = hi - (kh - PAD)
            if 0 <= ho < H:
                T[:, hi, :, ho] = weight[:, kh, :]
    return T


def _build_nc() -> bass.Bass:
    """Raw-bass builder: explicit per-engine streams with standalone
    wait_ge instructions (walrus only fits 2 sync commands — waits plus
    updates — per TPB instruction, so waits must be their own queue
    entries; every DMA/compute instruction carries at most one then_inc).

    DMA completions on a queue are NOT ordered across in-flight DMAs, so
    each ring slot / wall chunk gets its OWN semaphore: a wait then names
    exactly the transfer it needs.  Engine-sem increments (s_pe, s_cp) are
    engine-FIFO and stay single counters.

    Protocol (per channel c):
      SYNC:   [c>=XD] wait s_pe>=c-XD+1 ; dma x(c)->xring[c%XD]
              (+16 s_xs[c%XD]); wall preloaded in 8 chunks (+16 s_ws[wc])
      SCALAR: wait s_cp>=c+1 ; dma oring[c%OD]->out(c) (+16 s_outs[c%OD])
      PE:     [c%8==0] wait s_ws[c//8]>=16; wait s_xs[c%XD]>=16(c//XD+1)
              [c>=2] wait s_cp>=c-1 ; 28 matmuls into ps[c%2]; last +1 s_pe
      DVE:    wait s_pe>=c+1; [c>=OD] wait s_outs[c%OD]>=16(c//OD)
              copy ps[c%2] -> oring[c%OD] (f32->bf16), +1 s_cp
    """
    nc = bass.Bass()
    # host packs x as [c, h, n, w_pad] and wt as [h, c, kw, m]; out comes
    # back as [c, h, n, w] — all DMAs fully contiguous per partition row.
    xs = nc.declare_dram_parameter("xs", [CPC, H, N, WPAD], BF16, isOutput=False)
    wt = nc.declare_dram_parameter("wt", [H, CPC, KW, H], BF16, isOutput=False)
    out = nc.declare_dram_parameter("out", [CPC, H, N, W], BF16, isOutput=True)

    BANK = 512   # f32 elems per PSUM bank
    XD = 8       # x-ring depth
    OD = 8       # out-ring depth
    WCHUNK = 8   # channels per wall-preload chunk
    NWC = CPC // WCHUNK
    WCOLS = KW * H

    with ExitStack() as ctx:
        wall = ctx.enter_context(nc.sbuf_tensor("wall", [H, CPC * KW * H], BF16))
        xring = [
            ctx.enter_context(nc.sbuf_tensor(f"xb{i}", [H, N * WPAD], BF16))
            for i in range(XD)
        ]
        oring = [
            ctx.enter_context(nc.sbuf_tensor(f"ob{i}", [H, N * W], BF16))
            for i in range(OD)
        ]
        psums = [
            ctx.enter_context(nc.psum_tensor(f"ps{i}", [H, NB * BANK], F32))
            for i in range(2)
        ]
        s_xs = [ctx.enter_context(nc.semaphore(f"s_x{i}")) for i in range(XD)]
        s_ws = [ctx.enter_context(nc.semaphore(f"s_w{i}")) for i in range(NWC)]
        s_outs = [ctx.enter_context(nc.semaphore(f"s_o{i}")) for i in range(OD)]
        s_pe = ctx.enter_context(nc.semaphore("s_pe"))
        s_cp = ctx.enter_context(nc.semaphore("s_cp"))
        block = ctx.enter_context(nc.Block())

        @block.sync
        def _(sync):
            # interleave early x tiles with the wall chunks so PE has a
            # steady diet from channel 0 onward
            for cl in range(CPC):
                if cl >= XD:
                    sync.wait_ge(s_pe, cl - XD + 1)
                sync.dma_start(
                    xring[cl % XD][:], xs[cl].rearrange("p n w -> p (n w)")
                ).then_inc(s_xs[cl % XD], 16)
                if cl < NWC:
                    wc = cl
                    sync.dma_start(
                        wall[:, wc * WCHUNK * WCOLS:(wc + 1) * WCHUNK * WCOLS],
                        wt[:, wc * WCHUNK:(wc + 1) * WCHUNK].rearrange(
                            "h c k m -> h (c k m)"
                        ),
                    ).then_inc(s_ws[wc], 16)
            for i in range(OD):
                sync.wait_ge(s_outs[i], 16 * (CPC // OD))

        @block.scalar
        def _(scalar):
            for cl in range(CPC):
                scalar.wait_ge(s_cp, cl + 1)
                scalar.dma_start(
                    out[cl].rearrange("p n w -> p (n w)"), oring[cl % OD][:]
                ).then_inc(s_outs[cl % OD], 16)

        @block.tensor
        def _(tensor):
            wallv = wall[:].rearrange("p (c k m) -> p c k m", c=CPC, k=KW)
            for cl in range(CPC):
                if cl % WCHUNK == 0:
                    tensor.wait_ge(s_ws[cl // WCHUNK], 16)
                tensor.wait_ge(s_xs[cl % XD], 16 * (cl // XD + 1))
                if cl >= 2:
                    tensor.wait_ge(s_cp, cl - 1)
                xv = xring[cl % XD][:].rearrange("p (n w) -> p n w", w=WPAD)
                ps = psums[cl % 2]
                for nb in range(NB):
                    for kw in range(KW):
                        rhs = xv[:, nb * NCHUNK:(nb + 1) * NCHUNK, kw:kw + W]
                        mm = nc.tensor.matmul(
                            ps[:, nb * BANK:nb * BANK + FREE],
                            wallv[:, cl, kw, :],
                            rhs,
                            start=(kw == 0),
                            stop=(kw == KW - 1),
                        )
                mm.then_inc(s_pe, 1)

        @block.vector
        def _(vector):
            for cl in range(CPC):
                vector.wait_ge(s_pe, cl + 1)
                if cl >= OD:
                    vector.wait_ge(s_outs[cl % OD], 16 * (cl // OD))
                src = psums[cl % 2][:].rearrange("p (n b) -> p n b", b=BANK)[
                    :, :, 0:FREE
                ]
                dst = oring[cl % OD][:].rearrange("p (n w) -> p n w", w=FREE)
                nc.vector.tensor_copy(dst, src).then_inc(s_cp, 1)

    return nc


_NC_CACHE = None


def _get_nc() -> bass.Bass:
    global _NC_CACHE
    if _NC_CACHE is None:
        _NC_CACHE = _build_nc()
    return _NC_CACHE


def _run(x, codebook, bias, trace=False, **spmd_kwargs):
    weight = _hash_weight(np.asarray(codebook))
    T = _toeplitz_simple(weight).astype(NP_BF16)  # [C, H, KW, H]
    x = np.asarray(x)
    bias = np.asarray(bias).astype(np.float32)

    # pack x as [C, H, N, WPAD] so device DMA rows are contiguous
    x_bf = np.zeros((C, H, N, WPAD), dtype=NP_BF16)
    x_bf[:, :, :, PAD:PAD + W] = x.astype(NP_BF16).transpose(1, 2, 0, 3)
    in_maps = []
    for i in range(NCORES):
        c0 = i * CPC
        in_maps.append(
            {
                "xs": np.ascontiguousarray(x_bf[c0:c0 + CPC]),
                # wall layout [h, c, kw, m]
                "wt": np.ascontiguousarray(
                    T[c0:c0 + CPC].transpose(1, 0, 2, 3)
                ),
            }
        )

    nc = _get_nc()
    res = run_bass_kernel_spmd(
        nc, in_maps, core_ids=list(range(NCORES)), trace=trace, **spmd_kwargs
    )
    # device out is [CPC, H, N, W] bf16 per core -> [N, C, H, W] f32
    out = np.concatenate(
        [np.asarray(r["out"]) for r in res.results], axis=0
    ).transpose(2, 0, 1, 3).astype(np.float32)
    if np.any(bias):
        out = out + bias[None, :, None, None]
    return out, res


def kernel(x, codebook, bias):
    out, _ = _run(x, codebook, bias, trace=False)
    return out


if __name__ == "__main__":
    # smoke test: tiny numeric check of the toeplitz construction vs direct conv
    rng = np.random.default_rng(0)
    cb = (rng.standard_normal(CODEBOOK_SIZE) * 0.01).astype(np.float32)
    w = _hash_weight(cb)
    T = _toeplitz_simple(w)
    # direct conv of one channel, one image column
    xs = rng.standard_normal((C, H)).astype(np.float32)
    c = 123
    ref = np.zeros(H)
    for ho in range(H):
        for kh in range(KH):
            hi = ho + kh - PAD
            if 0 <= hi < H:
                ref[ho] += w[c, kh, 2] * xs[c, hi]
    got = T[c, :, 2, :].T @ xs[c]
    print("toeplitz check:", np.abs(ref - got).max())
